# revision 6
# baseline (speedup 1.0000x reference)
"""Behler-Parrinello NN on Trainium2: 8-core data-parallel Bass/Tile kernel.

Strategy
--------
- Shard the atom axis N across 8 cores (each core: 4 types x 16384 atoms).
- Host pre-transposes x to [T, F, Nshard] so features land on SBUF partitions;
  the whole MLP then runs on the PE with atoms on the moving/free axis
  (float32r matmuls, N=512 per matmul -> full-rate 4-byte path).
- Layer 4 ([1,100] @ h3) is computed as out = h3_chunk.T @ W4 per 128-atom
  group so the per-atom energies land with atoms on PSUM *partitions*.
- The scatter-add e[ind] += v is turned into matmuls: with m = q*128 + r,
  A[n,q] = v_n * (q_n == q), B[n,r] = (r_n == r)  =>  e_qr += A.T @ B,
  accumulated over the entire kernel in one persistent PSUM tile.
- Host sums the 8 per-core partial e grids.
"""

import os
from contextlib import ExitStack

import numpy as np

import concourse.bacc as bacc
import concourse.mybir as mybir
import concourse.tile as tile
from concourse.bass_utils import run_bass_kernel_spmd

F32 = mybir.dt.float32
F32R = mybir.dt.float32r
AF = mybir.ActivationFunctionType
ALU = mybir.AluOpType

T, F = 4, 128
H1, H2, H3 = 500, 200, 100
MOLS = 16384
NCORES = 8
NFULL = 131072
NSHARD = NFULL // NCORES  # 16384 atoms per type per core
BLK = 512                 # atoms per block (PE moving-dim / PSUM bank)
GPB = BLK // 128          # 128-atom groups per block

H1CS = 125  # H1 = 4 x 125
H1C = 4
H2CS = 100  # H2 = 2 x 100 (M chunks for layer 2)
H2C = 2
H3KC = 2    # layer-3 contraction 200 = 2 x 100


def build_program(ns=NSHARD, t_types=T):
    """Build and compile the single-core program (SPMD: same on all cores)."""
    assert ns % BLK == 0
    nblk = ns // BLK
    X = ns // 128

    nc = bacc.Bacc(
        "TRN2", target_bir_lowering=False, debug=False, enable_asserts=False
    )

    def din(name, shape):
        return nc.dram_tensor(name, shape, F32, kind="ExternalInput").ap()

    xT = din("xT", [t_types, F, ns])
    qT = din("qT", [t_types, 128, X])
    rT = din("rT", [t_types, 128, X])
    w1t = din("w1t", [t_types, F, H1])
    w2t = din("w2t", [t_types, H1CS, H1C * H2])
    w3t = din("w3t", [t_types, H3, H3KC * H3])
    w4t = din("w4t", [t_types, H3, 1])
    b1 = din("b1", [t_types, H1, 1])
    b2 = din("b2", [t_types, H2, 1])
    b3 = din("b3", [t_types, H3, 1])
    b4b = din("b4b", [t_types, 128, 1])
    iota = din("iota", [128, 128])
    eout = nc.dram_tensor("e_part", [128, 128], F32, kind="ExternalOutput").ap()

    n_scatter = t_types * nblk * GPB
    scnt = 0

    with tile.TileContext(nc) as tc:
        with ExitStack() as ctx:
            const = ctx.enter_context(tc.tile_pool(name="const", bufs=1))
            wpool = ctx.enter_context(tc.tile_pool(name="w", bufs=2))
            xpool = ctx.enter_context(tc.tile_pool(name="x", bufs=4))
            hpool = ctx.enter_context(tc.tile_pool(name="h", bufs=2))
            abpool = ctx.enter_context(tc.tile_pool(name="ab", bufs=4))
            vsbp = ctx.enter_context(tc.tile_pool(name="vsb", bufs=2))
            z1pool = ctx.enter_context(tc.tile_pool(name="z1", bufs=2, space="PSUM"))
            zpool = ctx.enter_context(tc.tile_pool(name="z", bufs=3, space="PSUM"))
            vpool = ctx.enter_context(tc.tile_pool(name="v", bufs=2, space="PSUM"))
            epool = ctx.enter_context(tc.tile_pool(name="e", bufs=1, space="PSUM"))

            iota_sb = const.tile([128, 128], F32, tag="iota")
            nc.sync.dma_start(iota_sb[:], iota[:])

            e_ps = epool.tile([128, 128], F32, tag="eacc")

            for t in range(t_types):
                w1_sb = wpool.tile([F, H1], F32R, tag="w1")
                nc.sync.dma_start(w1_sb[:], w1t[t].bitcast(F32R))
                w2_sb = wpool.tile([H1CS, H1C * H2], F32R, tag="w2")
                nc.sync.dma_start(w2_sb[:], w2t[t].bitcast(F32R))
                w3_sb = wpool.tile([H3, H3KC * H3], F32R, tag="w3")
                nc.sync.dma_start(w3_sb[:], w3t[t].bitcast(F32R))
                w4_sb = wpool.tile([H3, 1], F32R, tag="w4")
                nc.sync.dma_start(w4_sb[:], w4t[t].bitcast(F32R))
                b1_sb = wpool.tile([H1CS, H1C], F32, tag="b1")
                nc.sync.dma_start(
                    b1_sb[:], b1[t].rearrange("(c p) x -> p (c x)", c=H1C)
                )
                b2_sb = wpool.tile([H2CS, H2C], F32, tag="b2")
                nc.sync.dma_start(
                    b2_sb[:], b2[t].rearrange("(c p) x -> p (c x)", c=H2C)
                )
                b3_sb = wpool.tile([H3, 1], F32, tag="b3")
                nc.sync.dma_start(b3_sb[:], b3[t])
                b4_sb = wpool.tile([128, 1], F32, tag="b4")
                nc.sync.dma_start(b4_sb[:], b4b[t])
                q_sb = wpool.tile([128, X], F32, tag="q")
                nc.sync.dma_start(q_sb[:], qT[t])
                r_sb = wpool.tile([128, X], F32, tag="r")
                nc.sync.dma_start(r_sb[:], rT[t])

                for b in range(nblk):
                    xt = xpool.tile([128, BLK], F32R, tag="xt")
                    nc.sync.dma_start(
                        xt[:], xT[t, :, b * BLK:(b + 1) * BLK].bitcast(F32R)
                    )

                    # ---- layer 1: z1 = W1 @ x ; h1 = relu(z1 + b1) ----
                    h1 = hpool.tile([H1CS, H1C * BLK], F32R, tag="h1")
                    for c in range(H1C):
                        z1 = z1pool.tile([H1CS, BLK], F32, tag="z1")
                        nc.tensor.matmul(
                            z1[:],
                            lhsT=w1_sb[:, c * H1CS:(c + 1) * H1CS],
                            rhs=xt[:],
                            start=True,
                            stop=True,
                        )
                        dst = h1[:, c * BLK:(c + 1) * BLK]
                        if c < 2:
                            nc.scalar.activation(
                                dst, z1[:], AF.Relu, bias=b1_sb[:, c:c + 1]
                            )
                        else:
                            nc.vector.tensor_scalar(
                                dst, z1[:], b1_sb[:, c:c + 1], 0.0,
                                op0=ALU.add, op1=ALU.max,
                            )

                    # ---- layer 2: h2 = relu(W2 @ h1 + b2) ----
                    h2 = hpool.tile([H2CS, H2C * BLK], F32R, tag="h2")
                    for mc in range(H2C):
                        z2 = zpool.tile([H2CS, BLK], F32, tag="z")
                        for kc in range(H1C):
                            nc.tensor.matmul(
                                z2[:],
                                lhsT=w2_sb[
                                    :, kc * H2 + mc * H2CS: kc * H2 + (mc + 1) * H2CS
                                ],
                                rhs=h1[:, kc * BLK:(kc + 1) * BLK],
                                start=(kc == 0),
                                stop=(kc == H1C - 1),
                            )
                        dst = h2[:, mc * BLK:(mc + 1) * BLK]
                        if mc == 0:
                            nc.scalar.activation(
                                dst, z2[:], AF.Relu, bias=b2_sb[:, mc:mc + 1]
                            )
                        else:
                            nc.vector.tensor_scalar(
                                dst, z2[:], b2_sb[:, mc:mc + 1], 0.0,
                                op0=ALU.add, op1=ALU.max,
                            )

                    # ---- layer 3: h3 = relu(W3 @ h2 + b3) ----
                    h3 = hpool.tile([H3, BLK], F32R, tag="h3")
                    z3 = zpool.tile([H3, BLK], F32, tag="z")
                    for kc in range(H3KC):
                        nc.tensor.matmul(
                            z3[:],
                            lhsT=w3_sb[:, kc * H3:(kc + 1) * H3],
                            rhs=h2[:, kc * BLK:(kc + 1) * BLK],
                            start=(kc == 0),
                            stop=(kc == H3KC - 1),
                        )
                    nc.vector.tensor_scalar(
                        h3[:], z3[:], b3_sb[:], 0.0, op0=ALU.add, op1=ALU.max
                    )

                    # ---- layer 4 (transposed): v[n] = h3[:,n] . W4  ----
                    v_ps = vpool.tile([128, GPB], F32, tag="v")
                    for g in range(GPB):
                        nc.tensor.matmul(
                            v_ps[:, g:g + 1],
                            lhsT=h3[:, g * 128:(g + 1) * 128].bitcast(F32),
                            rhs=w4_sb[:].bitcast(F32),
                            start=True,
                            stop=True,
                        )
                    v_sb = vsbp.tile([128, GPB], F32, tag="vsb")
                    nc.scalar.activation(
                        v_sb[:], v_ps[:], AF.Identity, bias=b4_sb[:]
                    )

                    # ---- scatter-add as matmul: e[q,r] += sum_n v_n 1[q_n=q] 1[r_n=r]
                    for g in range(GPB):
                        col = b * GPB + g
                        a_sb = abpool.tile([128, 128], F32, tag="A")
                        nc.vector.tensor_scalar(
                            a_sb[:], iota_sb[:],
                            q_sb[:, col:col + 1], v_sb[:, g:g + 1],
                            op0=ALU.is_equal, op1=ALU.mult,
                        )
                        bt_sb = abpool.tile([128, 128], F32, tag="B")
                        nc.gpsimd.tensor_scalar(
                            bt_sb[:], iota_sb[:],
                            r_sb[:, col:col + 1], None,
                            op0=ALU.is_equal,
                        )
                        nc.tensor.matmul(
                            e_ps[:],
                            lhsT=a_sb[:],
                            rhs=bt_sb[:],
                            start=(scnt == 0),
                            stop=(scnt == n_scatter - 1),
                        )
                        scnt += 1

            e_sb = const.tile([128, 128], F32, tag="eout")
            nc.vector.tensor_copy(e_sb[:], e_ps[:])
            nc.sync.dma_start(eout, e_sb[:])

    nc.compile()
    return nc


def prep_shared(W1, b1, W2, b2, W3, b3, W4, b4):
    """Weight/bias layout marshaling (replicated across cores)."""
    f = np.float32
    w1t = np.ascontiguousarray(W1.transpose(0, 2, 1), dtype=f)          # [T,F,H1]
    w2t = np.ascontiguousarray(
        W2.transpose(0, 2, 1)
        .reshape(T, H1C, H1CS, H2)
        .transpose(0, 2, 1, 3)
        .reshape(T, H1CS, H1C * H2),
        dtype=f,
    )
    w3t = np.ascontiguousarray(
        W3.transpose(0, 2, 1)
        .reshape(T, H3KC, H3, H3)
        .transpose(0, 2, 1, 3)
        .reshape(T, H3, H3KC * H3),
        dtype=f,
    )
    w4t = np.ascontiguousarray(W4.transpose(0, 2, 1), dtype=f)          # [T,H3,1]
    out = {
        "w1t": w1t,
        "w2t": w2t,
        "w3t": w3t,
        "w4t": w4t,
        "b1": np.ascontiguousarray(b1.reshape(T, H1, 1), dtype=f),
        "b2": np.ascontiguousarray(b2.reshape(T, H2, 1), dtype=f),
        "b3": np.ascontiguousarray(b3.reshape(T, H3, 1), dtype=f),
        "b4b": np.ascontiguousarray(
            np.broadcast_to(b4.reshape(T, 1, 1), (T, 128, 1)), dtype=f
        ),
        "iota": np.ascontiguousarray(
            np.broadcast_to(np.arange(128, dtype=f), (128, 128))
        ),
    }
    return out


def prep_core(x, ind, core, ns=NSHARD):
    """Per-core shard marshaling: transposed x and split/transposed indices."""
    f = np.float32
    sl = slice(core * ns, (core + 1) * ns)
    X = ns // 128
    xs = x[:, sl, :]
    xT = np.ascontiguousarray(xs.transpose(0, 2, 1), dtype=f)           # [T,F,ns]
    inds = np.asarray(ind[:, sl])
    q = (inds // 128).astype(f)
    r = (inds % 128).astype(f)
    qT = np.ascontiguousarray(q.reshape(T, X, 128).transpose(0, 2, 1))  # [T,128,X]
    rT = np.ascontiguousarray(r.reshape(T, X, 128).transpose(0, 2, 1))
    return {"xT": xT, "qT": qT, "rT": rT}


_CACHE = {}


def _get_program():
    if "nc" not in _CACHE:
        _CACHE["nc"] = build_program()
    return _CACHE["nc"]


def _ensure_ntff_hook():
    """Install the axon NTFF profile hook if the image's antenv lacks it."""
    import sys
    import types

    try:
        from antenv.axon_hooks import get_axon_ntff_profile_hook  # noqa: F401
        return
    except ImportError:
        pass
    try:
        from trn_agent_boot.trn_boot import _ntff_profile_via_ctypes
    except ImportError:
        return
    so = "/opt/axon/libaxon_pjrt.so"
    if not os.path.exists(so):
        return
    hook = _ntff_profile_via_ctypes(so)
    mod = types.ModuleType("antenv.axon_hooks")
    state = {"hook": hook}
    mod.get_axon_ntff_profile_hook = lambda: state["hook"]
    mod.set_axon_ntff_profile_hook = lambda h: state.update(hook=h)
    sys.modules["antenv.axon_hooks"] = mod


def run(inputs, trace=False, trace_kwargs=None):
    """Run the 8-core kernel. Returns (out [M,1] f32, BassKernelResults)."""
    x = np.asarray(inputs["x"], dtype=np.float32)
    ind = np.asarray(inputs["ind"])
    e = np.asarray(inputs["e"], dtype=np.float32)
    shared = prep_shared(
        np.asarray(inputs["W1"]), np.asarray(inputs["b1"]),
        np.asarray(inputs["W2"]), np.asarray(inputs["b2"]),
        np.asarray(inputs["W3"]), np.asarray(inputs["b3"]),
        np.asarray(inputs["W4"]), np.asarray(inputs["b4"]),
    )
    in_maps = []
    for c in range(NCORES):
        m = dict(shared)
        m.update(prep_core(x, ind, c))
        in_maps.append(m)

    nc = _get_program()
    if trace:
        _ensure_ntff_hook()
    res = run_bass_kernel_spmd(
        nc,
        in_maps,
        core_ids=list(range(NCORES)),
        trace=trace,
        **(trace_kwargs or {}),
    )
    acc = e.reshape(-1).astype(np.float64).copy()
    for rm in res.results:
        acc += rm["e_part"].astype(np.float64).reshape(-1)
    out = acc.astype(np.float32).reshape(MOLS, 1)
    return out, res


def kernel(**inputs):
    out, _ = run(inputs, trace=False)
    return out
